# revision 20
# baseline (speedup 1.0000x reference)
"""Multi-head attention (B=2, N=4096, D=768, H=8) on 8 trn2 NeuronCores.

Sharding: core c handles batch b = c//4 and head-pair hp = c%4 (heads 2hp,
2hp+1).  Each core computes the qkv projection for its 2 heads plus full
4096x4096 attention for them; no cross-core communication.

Device-side layout (per core):
  xT    [768, 4096] fp16   x[b] transposed (host-prepped)
  wqk   [768, 384]  fp16   [Wq_h0 | Wq_h1 | Wk_h0 | Wk_h1]  (unscaled)
  bqk96 [96, 4]     fp32   matching biases as per-partition columns
  wv    [768, 194]  fp16   [Wv_h0 | 0 | Wv_h1 | 0]
  wvaug [1, 194]    fp16   [bv_h0 | 1 | bv_h1 | 1]  (ones row of aug x)
  out   [2, 8, 97, 512] fp32  [h, window, dim|den, token-in-window]

Design notes (all constants HW-measured via mmbench.py loop-slope):
- Scores S^T[key, query] = kT.T @ qT per 128-key tile, FD=512 fp16:
  213.4 ns/MM (pure 1 col/cycle streaming; ldweights hidden).  fp8
  DoubleRow measured 424 ns/MM for the same shape (no double-pump on this
  hw + unhidden 256-col ldweights) -- fp16 is the right precision.
- PV is REVERSED vs the obvious layout: out[dim, query] accumulates with
  lhsT = V-tile [128 keys, 97] stationary and the exp tile [128 keys,
  512 queries] moving: ONE 197.1 ns matmul per key tile instead of four
  97-wide ones (4 x 65.4 = 262 ns) -- ldweights (97 cols) hides under the
  512-col stream.  Column 96 of V carries the all-ones softmax-denominator
  column; normalization happens on the host in gather_out.
- Score tiles rotate over five single-bank PSUM slots; each tile gets
  ONE exp op on alternating engines: even tiles exact Exp on ScalarE
  (written through a bitcast fp16 view of an int16 tile), odd tiles a
  one-instruction Schraudolph fast-exp on VectorE (s*EA'+EB -> int16 =
  fp16 bits, ~1.7% rms weight error, largely cancelling in softmax
  normalization).  Per-tile exp fires right after its own score matmul,
  so the score->exp->slot-release chain (~1.6 us on hw) hides well under
  the 5-tile (~2.7 us) slot reuse distance.  Fast-exp fraction 0.5 ->
  rel err ~9.2e-3 vs the 2e-2 budget.
- Projection epilogues, V copies and window out-copies all ride VectorE;
  ScalarE does nothing but exp.  Projections for later windows/heads are
  spread through the attention stream (FILLER_MOD) as PE slack filler.
"""

import sys

for _p in ("/opt/trn_rl_repo",):
    if _p not in sys.path:
        sys.path.insert(0, _p)

import numpy as np

B = 2
N = 4096
DIM = 768
H = 8
DH = 96
SCALE = DIM ** -0.5
NCORES = 8
VW = 2 * DH + 2  # 194: [v_h0 | ones | v_h1 | ones]
NT = N // 128    # 32 token tiles
NBLK = N // 512  # 8 blocks of 512
DT = DIM // 128  # 6 contraction tiles

_CACHE = {}
PVLAG = 6        # key tiles of score->PV lag
EXBUFS = 6       # exp tiles in flight (per slot tag)
FILLER_MOD = 22  # spread the 23 filler projections over all ~506 PV pops
VLOOK = 6
KLOOK = 3

# Schraudolph fast-exp on DVE: bits_f16(exp(s*SCALE)) ~= int16(s*EA'+EB).
# EA' folds SCALE; EB = 1024*15 - 45 (bias tuned) + 0.5 (int16 convert
# truncates toward zero; inputs keep y positive).  BMEAN matches the exact
# exp's mean to the Schraudolph family's (ratio 1.00932).
EA = 1024.0 / float(np.log(2.0)) * SCALE
EB = 1024.0 * 15 - 45.0 + 0.5
ASPLIT = 896     # pair-exp split: ACT exact on [0:896), DVE on [896:1024)
BMEAN = float(np.log(1.00932))


def build_program(loop_iters=1, variant="full"):
    import concourse.tile as tile
    from concourse import bacc, mybir

    F16 = mybir.dt.float16
    F32 = mybir.dt.float32
    I16 = mybir.dt.int16
    Exp = mybir.ActivationFunctionType.Exp
    Mult = mybir.AluOpType.mult
    Add = mybir.AluOpType.add

    nc = bacc.Bacc("TRN2", target_bir_lowering=False, debug=False)
    xT_h = nc.declare_dram_parameter("xT", [DIM, N], F16, isOutput=False)
    wqk_h = nc.declare_dram_parameter("wqk", [DIM, 4 * DH], F16, isOutput=False)
    bqk96_h = nc.declare_dram_parameter("bqk96", [DH, 4], F32, isOutput=False)
    wv_h = nc.declare_dram_parameter("wv", [DIM, VW], F16, isOutput=False)
    wvaug_h = nc.declare_dram_parameter("wvaug", [1, VW], F16, isOutput=False)
    # out[h, nw, d, c]: d<96 = UNNORMALIZED numerator dim d, d=96 = softmax
    # denominator, for token nw*512 + c of head h.
    out_h = nc.declare_dram_parameter(
        "out", [2, NBLK, DH + 1, 512], F32, isOutput=True
    )

    xT, wqk, bqk96 = xT_h.ap(), wqk_h.ap(), bqk96_h.ap()
    wv, wvaug, out = wv_h.ap(), wvaug_h.ap(), out_h.ap()

    with tile.TileContext(nc) as tc:
        with (
            tc.tile_pool(name="const", bufs=1) as const,
            tc.tile_pool(name="work", bufs=3) as work,
            tc.tile_pool(name="pp", bufs=2, space="PSUM") as pp,
        ):
            # --- persistent SBUF tensors ---
            xt_sb = [
                const.tile([128, N], F16, name=f"xt{d}", tag=f"xt{d}")
                for d in range(DT)
            ]
            wqk_sb = [
                const.tile([128, 4 * DH], F16, name=f"wqksb{d}", tag=f"wqksb{d}")
                for d in range(DT)
            ]
            wv_sb = [
                const.tile([128, VW], F16, name=f"wvsb{d}", tag=f"wvsb{d}")
                for d in range(DT)
            ]
            wvaug_sb = const.tile([1, VW], F16, name="wvaug_sb")
            bqk96_sb = const.tile([DH, 4], F32, name="bqk96_sb")
            ones_sb = const.tile([1, 128], F16, name="ones_sb")
            bm_sb = const.tile([128, 1], F32, name="bm_sb")
            qkT_sb = [
                const.tile([DH, N], F16, name=f"qkT{j}", tag=f"qkT{j}")
                for j in range(4)
            ]
            v_sb = const.tile([128, NT * VW], F16, name="v_sb")

            nc.sync.dma_start(out=bqk96_sb, in_=bqk96)
            nc.sync.dma_start(out=wvaug_sb, in_=wvaug)
            for d in range(DT):
                nc.sync.dma_start(out=wqk_sb[d], in_=wqk[d * 128:(d + 1) * 128, :])
            # xT arrives in column chunks, in the order the first attention
            # window consumes them; V weights slot in after the first chunk.
            for blk in range(NBLK):
                for d in range(DT):
                    nc.sync.dma_start(
                        out=xt_sb[d][:, blk * 512:(blk + 1) * 512],
                        in_=xT[d * 128:(d + 1) * 128, blk * 512:(blk + 1) * 512],
                    )
                if blk == 0:
                    for d in range(DT):
                        nc.sync.dma_start(
                            out=wv_sb[d], in_=wv[d * 128:(d + 1) * 128, :]
                        )
            nc.vector.memset(ones_sb, 1.0)
            nc.vector.memset(bm_sb, BMEAN)

            qk_done = set()
            v_done = set()

            def ensure_qk(j, blk):
                # qkT[j][:, blk] = (wqk[:, j] block).T @ xT[:, blk] + bias_j
                if (j, blk) in qk_done:
                    return
                qk_done.add((j, blk))
                pt = pp.tile([DH, 512], F32, tag="pj", name="pt", bufs=2)
                for d in range(DT):
                    nc.tensor.matmul(
                        pt,
                        lhsT=wqk_sb[d][:, j * DH:(j + 1) * DH],
                        rhs=xt_sb[d][:, blk * 512:(blk + 1) * 512],
                        start=(d == 0),
                        stop=(d == DT - 1),
                    )
                nc.vector.tensor_scalar_add(
                    out=qkT_sb[j][:, blk * 512:(blk + 1) * 512],
                    in0=pt,
                    scalar1=bqk96_sb[:, j:j + 1],
                )

            def ensure_v(t):
                if t in v_done:
                    return
                v_done.add(t)
                pv = pp.tile([128, VW], F32, tag="pj", name="pv", bufs=2)
                for d in range(DT):
                    nc.tensor.matmul(
                        pv,
                        lhsT=xt_sb[d][:, t * 128:(t + 1) * 128],
                        rhs=wv_sb[d],
                        start=(d == 0),
                        stop=False,
                    )
                nc.tensor.matmul(
                    pv,
                    lhsT=ones_sb,
                    rhs=wvaug_sb,
                    start=False,
                    stop=True,
                )
                nc.vector.tensor_copy(out=v_sb[:, t * VW:(t + 1) * VW], in_=pv)

            # filler: projection units to slip into PE slack inside the
            # attention stream, ordered by deadline.
            filler = []
            for b in range(1, NBLK):
                filler.append((0, b))       # q_h0 blk b: before window (0, b)
                filler.append((3, b - 1))   # k_h1: all before head 1
            filler.append((3, NBLK - 1))
            for b in range(NBLK):
                filler.append((1, b))       # q_h1 blk b: before window (1, b)
            fill_state = {"i": 0, "tick": 0}

            def pop_filler():
                fill_state["tick"] += 1
                if fill_state["tick"] % FILLER_MOD == 0 and fill_state["i"] < len(filler):
                    j, b = filler[fill_state["i"]]
                    fill_state["i"] += 1
                    ensure_qk(j, b)

            # software pipeline: PV lags scores by PVLAG key tiles; the queue
            # carries ACROSS window boundaries so the pipeline never drains.
            pending = []  # tiles with exp emitted, awaiting PV

            def emit_pv(hh, ww, pva_, mt, ex16, base):
                if variant != "nopv":
                    # reversed PV: V stationary, exp slab moving, out [97, 512]
                    nc.tensor.matmul(
                        pva_,
                        lhsT=v_sb[:, mt * VW + hh * 97:mt * VW + hh * 97 + 97],
                        rhs=ex16[:, base:base + 512],
                        start=(mt == 0),
                        stop=(mt == NT - 1),
                        skip_group_check=True,
                    )
                if mt == NT - 1:
                    # window ww finished accumulating: stage + DMA it out
                    ob = work.tile([DH + 1, 512], F32, tag="ob", name="ob",
                                   bufs=3)
                    nc.vector.tensor_copy(out=ob, in_=pva_)
                    nc.sync.dma_start(out=out[hh, ww], in_=ob)

            def attn_nw_stream():
                # Flat stream over all (h, nw, mt): score tiles rotate over
                # five single-bank PSUM slots (scA/scB [128,1024] pair tiles
                # + scS [128,512] solo).  Slots 0+1 / 2+3 are exp'd by one
                # wide op pair split across ACT+DVE; slot 4 all-DVE.
                state = {"pva": None}

                def one_tile(idx, h, nw, mt):
                    p = idx % 5
                    if mt == 0:
                        ensure_qk(h, nw)
                        state["pva"] = pp.tile([DH + 1, 512], F32, tag="pva",
                                               name="pva", bufs=1)
                    pva = state["pva"]
                    for b in range(mt * 128 // 512 + 1):
                        ensure_qk(2 + h, b)
                    if len(pending) >= PVLAG:
                        emit_pv(*pending.pop(0))
                        pop_filler()
                    sc = pp.tile([128, 512], F32, name="sc",
                                 tag=f"sc{p}", bufs=1)
                    nc.tensor.matmul(
                        sc,
                        lhsT=qkT_sb[2 + h][:, mt * 128:(mt + 1) * 128],
                        rhs=qkT_sb[h][:, nw * 512:(nw + 1) * 512],
                        start=True,
                        stop=True,
                    )
                    # per-tile exp on alternating engines: even tiles exact
                    # Exp on ACT (through a bitcast fp16 view), odd tiles
                    # Schraudolph on DVE.  Same 5-slot PSUM layout as the
                    # pair-exp version; only the exp granularity changes.
                    ex = work.tile([128, 512], I16,
                                   tag=f"ex{p}", name="ex", bufs=EXBUFS)
                    ex16 = ex.bitcast(F16)
                    if idx % 2 == 0:
                        nc.scalar.activation(
                            out=(ex16 if variant != "noexp" else ex16[:, :8]),
                            in_=(sc if variant != "noexp" else sc[:, :8]),
                            func=Exp, bias=bm_sb, scale=SCALE)
                    else:
                        nc.vector.tensor_scalar(
                            out=(ex if variant != "noexp" else ex[:, :8]),
                            in0=(sc if variant != "noexp" else sc[:, :8]),
                            scalar1=EA, scalar2=EB, op0=Mult, op1=Add)
                    pending.append((h, nw, pva, mt, ex16, 0))
                    # look-ahead projections/V AFTER the exp so their engine
                    # tails don't delay the exp delivery
                    for b in range(min(mt + KLOOK, NT - 1) * 128 // 512 + 1):
                        ensure_qk(2 + h, b)
                    for t in range(mt, min(mt + VLOOK, NT)):
                        ensure_v(t)

                idx = 0
                for h in range(2):
                    for nw in range(NBLK):
                        for mt in range(NT):
                            one_tile(idx, h, nw, mt)
                            idx += 1

            def body(_i=None):
                qk_done.clear()
                v_done.clear()
                fill_state["i"] = 0
                fill_state["tick"] = 0
                pending.clear()
                attn_nw_stream()
                for p in pending:
                    emit_pv(*p)
                    pop_filler()
                pending.clear()
                # backstop: anything the filler didn't reach
                for j, b in filler:
                    ensure_qk(j, b)

            if loop_iters == 1:
                body()
            else:
                with tc.For_i(0, loop_iters, 1) as _i:
                    body(_i)

    nc.compile()
    return nc


def get_program(loop_iters=1, variant="full"):
    key = ("nc", loop_iters, variant)
    if key not in _CACHE:
        _CACHE[key] = build_program(loop_iters, variant)
    return _CACHE[key]


def make_in_maps(x, W_qkv, b_qkv):
    x = np.asarray(x, np.float32)
    W = np.asarray(W_qkv, np.float32)
    b = np.asarray(b_qkv, np.float32)
    Wq, Wk, Wv = W[:, :DIM], W[:, DIM:2 * DIM], W[:, 2 * DIM:]
    bq, bk, bv = b[:DIM], b[DIM:2 * DIM], b[2 * DIM:]

    in_maps = []
    for c in range(NCORES):
        bb, hp = divmod(c, 4)
        h0 = 2 * hp
        s = slice(h0 * DH, (h0 + 1) * DH)
        s1 = slice((h0 + 1) * DH, (h0 + 2) * DH)
        xT = np.ascontiguousarray(x[bb].T).astype(np.float16)
        wqk = np.concatenate(
            [Wq[:, s], Wq[:, s1], Wk[:, s], Wk[:, s1]], axis=1
        ).astype(np.float16)
        bqk96 = np.stack([bq[s], bq[s1], bk[s], bk[s1]], axis=1).astype(
            np.float32
        )
        wv = np.zeros((DIM, VW), np.float16)
        wv[:, 0:DH] = Wv[:, s].astype(np.float16)
        wv[:, DH + 1:2 * DH + 1] = Wv[:, s1].astype(np.float16)
        wvaug = np.zeros((1, VW), np.float16)
        wvaug[0, 0:DH] = bv[s].astype(np.float16)
        wvaug[0, DH] = 1.0
        wvaug[0, DH + 1:2 * DH + 1] = bv[s1].astype(np.float16)
        wvaug[0, 2 * DH + 1] = 1.0
        in_maps.append(
            {"xT": xT, "wqk": wqk, "bqk96": bqk96, "wv": wv, "wvaug": wvaug}
        )
    return in_maps


def gather_out(results):
    out = np.empty((B, N, DIM), np.float32)
    for c in range(NCORES):
        bb, hp = divmod(c, 4)
        o = np.asarray(results[c]["out"], np.float32)  # [2, NBLK, 97, 512]
        # out[h, nw, d, c] = numerator dim d (d<96) / denominator (d=96)
        # for token nw*512 + c
        num = o[:, :, :DH, :]                 # [2, NBLK, 96, 512]
        den = o[:, :, DH:DH + 1, :]           # [2, NBLK, 1, 512]
        r = (num / den).transpose(0, 1, 3, 2).reshape(2, N, DH)
        out[bb, :, (2 * hp) * DH:(2 * hp + 1) * DH] = r[0]
        out[bb, :, (2 * hp + 1) * DH:(2 * hp + 2) * DH] = r[1]
    return out


def run(x, W_qkv, b_qkv, trace=False, **kw):
    from concourse.bass_utils import run_bass_kernel_spmd

    nc = get_program()
    in_maps = make_in_maps(x, W_qkv, b_qkv)
    res = run_bass_kernel_spmd(nc, in_maps, list(range(NCORES)), trace=trace, **kw)
    return gather_out(res.results), res


def kernel(x, W_qkv, b_qkv):
    out, _ = run(x, W_qkv, b_qkv)
    return out


# revision 21
# speedup vs baseline: 1.1377x; 1.1377x over previous
"""Multi-head attention (B=2, N=4096, D=768, H=8) on 8 trn2 NeuronCores.

Sharding: core c handles batch b = c//4 and head-pair hp = c%4 (heads 2hp,
2hp+1).  Each core computes the qkv projection for its 2 heads plus full
4096x4096 attention for them; no cross-core communication.

Device-side layout (per core):
  xT    [768, 4096] fp16   x[b] transposed (host-prepped)
  wqk   [768, 384]  fp16   [Wq_h0 | Wq_h1 | Wk_h0 | Wk_h1]  (unscaled)
  bqk96 [96, 4]     fp32   matching biases as per-partition columns
  wv    [768, 194]  fp16   [Wv_h0 | 0 | Wv_h1 | 0]  (bv added on host)
  out   [2, 8, 97, 512] fp32  [h, window, dim|den, token-in-window]

Design notes (all constants HW-measured via mmbench.py loop-slope):
- Scores S^T[key, query] = kT.T @ qT per 128-key tile, FD=512 fp16:
  213.4 ns/MM (pure 1 col/cycle streaming; ldweights hidden).  fp8
  DoubleRow measured 424 ns/MM for the same shape (no double-pump on this
  hw + unhidden 256-col ldweights) -- fp16 is the right precision.
- PV is REVERSED vs the obvious layout: out[dim, query] accumulates with
  lhsT = V-tile [128 keys, 97] stationary and the exp tile [128 keys,
  512 queries] moving: ONE 197.1 ns matmul per key tile instead of four
  97-wide ones (4 x 65.4 = 262 ns) -- ldweights (97 cols) hides under the
  512-col stream.  Column 96 of V carries the all-ones softmax-denominator
  column; normalization happens on the host in gather_out.
- Score tiles rotate over five single-bank PSUM slots; each tile gets
  ONE exp op on alternating engines: even tiles exact Exp on ScalarE
  (written through a bitcast fp16 view of an int16 tile), odd tiles a
  one-instruction Schraudolph fast-exp on VectorE (s*EA'+EB -> int16 =
  fp16 bits, ~1.7% rms weight error, largely cancelling in softmax
  normalization).  Per-tile exp fires right after its own score matmul,
  so the score->exp->slot-release chain (~1.6 us on hw) hides well under
  the 5-tile (~2.7 us) slot reuse distance.  Fast-exp fraction 0.5 ->
  rel err ~9.2e-3 vs the 2e-2 budget.
- Projection epilogues, V copies and window out-copies all ride VectorE;
  ScalarE does nothing but exp.  Projections for later windows/heads are
  spread through the attention stream (FILLER_MOD) as PE slack filler.
"""

import sys

for _p in ("/opt/trn_rl_repo",):
    if _p not in sys.path:
        sys.path.insert(0, _p)

import numpy as np

B = 2
N = 4096
DIM = 768
H = 8
DH = 96
SCALE = DIM ** -0.5
NCORES = 8
VW = 2 * DH + 2  # 194: [v_h0 | ones | v_h1 | ones]
NT = N // 128    # 32 token tiles
NBLK = N // 512  # 8 blocks of 512
DT = DIM // 128  # 6 contraction tiles

_CACHE = {}
PVLAG = 6        # key tiles of score->PV lag
EXBUFS = 6       # exp tiles in flight (per slot tag)
FILLER_MOD = 22  # spread the 23 filler projections over all ~506 PV pops
VLOOK = 6
KLOOK = 3

# Schraudolph fast-exp on DVE: bits_f16(exp(s*SCALE)) ~= int16(s*EA'+EB).
# EA' folds SCALE; EB = 1024*15 - 45 (bias tuned) + 0.5 (int16 convert
# truncates toward zero; inputs keep y positive).  BMEAN matches the exact
# exp's mean to the Schraudolph family's (ratio 1.00932).
EA = 1024.0 / float(np.log(2.0)) * SCALE
EB = 1024.0 * 15 - 45.0 + 0.5
ASPLIT = 896     # pair-exp split: ACT exact on [0:896), DVE on [896:1024)
BMEAN = float(np.log(1.00932))


def build_program(loop_iters=1, variant="full"):
    import concourse.tile as tile
    from concourse import bacc, mybir

    F16 = mybir.dt.float16
    F32 = mybir.dt.float32
    I16 = mybir.dt.int16
    Exp = mybir.ActivationFunctionType.Exp
    Mult = mybir.AluOpType.mult
    Add = mybir.AluOpType.add

    nc = bacc.Bacc("TRN2", target_bir_lowering=False, debug=False)
    xT_h = nc.declare_dram_parameter("xT", [DIM, N], F16, isOutput=False)
    wqk_h = nc.declare_dram_parameter("wqk", [DIM, 4 * DH], F16, isOutput=False)
    bqk96_h = nc.declare_dram_parameter("bqk96", [DH, 4], F32, isOutput=False)
    wv_h = nc.declare_dram_parameter("wv", [DIM, VW], F16, isOutput=False)
    # out[h, nw, d, c]: d<96 = UNNORMALIZED numerator dim d, d=96 = softmax
    # denominator, for token nw*512 + c of head h.
    out_h = nc.declare_dram_parameter(
        "out", [2, NBLK, DH + 1, 512], F32, isOutput=True
    )

    xT, wqk, bqk96 = xT_h.ap(), wqk_h.ap(), bqk96_h.ap()
    wv, out = wv_h.ap(), out_h.ap()

    with tile.TileContext(nc) as tc:
        with (
            tc.tile_pool(name="const", bufs=1) as const,
            tc.tile_pool(name="work", bufs=3) as work,
            tc.tile_pool(name="pp", bufs=2, space="PSUM") as pp,
        ):
            # --- persistent SBUF tensors ---
            xt_sb = [
                const.tile([128, N], F16, name=f"xt{d}", tag=f"xt{d}")
                for d in range(DT)
            ]
            wqk_sb = [
                const.tile([128, 4 * DH], F16, name=f"wqksb{d}", tag=f"wqksb{d}")
                for d in range(DT)
            ]
            wv_sb = [
                const.tile([128, VW], F16, name=f"wvsb{d}", tag=f"wvsb{d}")
                for d in range(DT)
            ]
            bqk96_sb = const.tile([DH, 4], F32, name="bqk96_sb")
            bm_sb = const.tile([128, 1], F32, name="bm_sb")
            qkT_sb = [
                const.tile([DH, N], F16, name=f"qkT{j}", tag=f"qkT{j}")
                for j in range(4)
            ]
            v_sb = const.tile([128, NT * VW], F16, name="v_sb")

            nc.sync.dma_start(out=bqk96_sb, in_=bqk96)
            for d in range(DT):
                nc.sync.dma_start(out=wqk_sb[d], in_=wqk[d * 128:(d + 1) * 128, :])
            # xT arrives in column chunks, in the order the first attention
            # window consumes them; V weights slot in after the first chunk.
            for blk in range(NBLK):
                for d in range(DT):
                    nc.sync.dma_start(
                        out=xt_sb[d][:, blk * 512:(blk + 1) * 512],
                        in_=xT[d * 128:(d + 1) * 128, blk * 512:(blk + 1) * 512],
                    )
                if blk == 0:
                    for d in range(DT):
                        nc.sync.dma_start(
                            out=wv_sb[d], in_=wv[d * 128:(d + 1) * 128, :]
                        )
            nc.vector.memset(bm_sb, BMEAN)
            for t in range(NT):
                nc.gpsimd.memset(v_sb[:, t * VW + DH:t * VW + DH + 1], 1.0)
                nc.gpsimd.memset(
                    v_sb[:, t * VW + 2 * DH + 1:t * VW + 2 * DH + 2], 1.0)

            qk_done = set()
            v_done = set()

            def ensure_qk(j, blk):
                # qkT[j][:, blk] = (wqk[:, j] block).T @ xT[:, blk] + bias_j
                if (j, blk) in qk_done:
                    return
                qk_done.add((j, blk))
                pt = pp.tile([DH, 512], F32, tag="pj", name="pt", bufs=2)
                for d in range(DT):
                    nc.tensor.matmul(
                        pt,
                        lhsT=wqk_sb[d][:, j * DH:(j + 1) * DH],
                        rhs=xt_sb[d][:, blk * 512:(blk + 1) * 512],
                        start=(d == 0),
                        stop=(d == DT - 1),
                    )
                nc.vector.tensor_scalar_add(
                    out=qkT_sb[j][:, blk * 512:(blk + 1) * 512],
                    in0=pt,
                    scalar1=bqk96_sb[:, j:j + 1],
                )

            def ensure_v(t):
                if t in v_done:
                    return
                v_done.add(t)
                pv = pp.tile([128, VW], F32, tag="pj", name="pv", bufs=2)
                for d in range(DT):
                    nc.tensor.matmul(
                        pv,
                        lhsT=xt_sb[d][:, t * 128:(t + 1) * 128],
                        rhs=wv_sb[d],
                        start=(d == 0),
                        stop=(d == DT - 1),
                    )
                nc.vector.tensor_copy(
                    out=v_sb[:, t * VW:t * VW + DH], in_=pv[:, :DH])
                nc.vector.tensor_copy(
                    out=v_sb[:, t * VW + DH + 1:t * VW + 2 * DH + 1],
                    in_=pv[:, DH + 1:2 * DH + 1])

            # filler: projection units to slip into PE slack inside the
            # attention stream, ordered by deadline.
            filler = []
            for b in range(1, NBLK):
                filler.append((0, b))       # q_h0 blk b: before window (0, b)
                filler.append((3, b - 1))   # k_h1: all before head 1
            filler.append((3, NBLK - 1))
            for b in range(NBLK):
                filler.append((1, b))       # q_h1 blk b: before window (1, b)
            fill_state = {"i": 0, "tick": 0}

            def pop_filler():
                fill_state["tick"] += 1
                if fill_state["tick"] % FILLER_MOD == 0 and fill_state["i"] < len(filler):
                    j, b = filler[fill_state["i"]]
                    fill_state["i"] += 1
                    ensure_qk(j, b)

            # software pipeline: PV lags scores by PVLAG key tiles; the queue
            # carries ACROSS window boundaries so the pipeline never drains.
            pending = []  # tiles with exp emitted, awaiting PV

            def emit_pv(hh, ww, pva_, mt, ex16, base):
                if variant != "nopv":
                    # reversed PV: V stationary, exp slab moving, out [97, 512]
                    nc.tensor.matmul(
                        pva_,
                        lhsT=v_sb[:, mt * VW + hh * 97:mt * VW + hh * 97 + 97],
                        rhs=ex16[:, base:base + 512],
                        start=(mt == 0),
                        stop=(mt == NT - 1),
                        skip_group_check=True,
                    )
                if mt == NT - 1:
                    # window ww finished accumulating: stage + DMA it out
                    ob = work.tile([DH + 1, 512], F32, tag="ob", name="ob",
                                   bufs=3)
                    nc.vector.tensor_copy(out=ob, in_=pva_)
                    nc.sync.dma_start(out=out[hh, ww], in_=ob)

            def attn_nw_stream():
                # Flat stream over all (h, nw, mt): score tiles rotate over
                # five single-bank PSUM slots (scA/scB [128,1024] pair tiles
                # + scS [128,512] solo).  Slots 0+1 / 2+3 are exp'd by one
                # wide op pair split across ACT+DVE; slot 4 all-DVE.
                state = {"pva": None}

                def one_tile(idx, h, nw, mt):
                    p = idx % 5
                    if mt == 0:
                        ensure_qk(h, nw)
                        state["pva"] = pp.tile([DH + 1, 512], F32, tag="pva",
                                               name="pva", bufs=1)
                    pva = state["pva"]
                    for b in range(mt * 128 // 512 + 1):
                        ensure_qk(2 + h, b)
                    if len(pending) >= PVLAG:
                        emit_pv(*pending.pop(0))
                        pop_filler()
                    sc = pp.tile([128, 512], F32, name="sc",
                                 tag=f"sc{p}", bufs=1)
                    nc.tensor.matmul(
                        sc,
                        lhsT=qkT_sb[2 + h][:, mt * 128:(mt + 1) * 128],
                        rhs=qkT_sb[h][:, nw * 512:(nw + 1) * 512],
                        start=True,
                        stop=True,
                    )
                    # per-tile exp on alternating engines: even tiles exact
                    # Exp on ACT (through a bitcast fp16 view), odd tiles
                    # Schraudolph on DVE.  Same 5-slot PSUM layout as the
                    # pair-exp version; only the exp granularity changes.
                    ex = work.tile([128, 512], I16,
                                   tag=f"ex{p}", name="ex", bufs=EXBUFS)
                    ex16 = ex.bitcast(F16)
                    if idx % 2 == 0:
                        nc.scalar.activation(
                            out=(ex16 if variant != "noexp" else ex16[:, :8]),
                            in_=(sc if variant != "noexp" else sc[:, :8]),
                            func=Exp, bias=bm_sb, scale=SCALE)
                    else:
                        nc.vector.tensor_scalar(
                            out=(ex if variant != "noexp" else ex[:, :8]),
                            in0=(sc if variant != "noexp" else sc[:, :8]),
                            scalar1=EA, scalar2=EB, op0=Mult, op1=Add)
                    pending.append((h, nw, pva, mt, ex16, 0))
                    # look-ahead projections/V AFTER the exp so their engine
                    # tails don't delay the exp delivery
                    for b in range(min(mt + KLOOK, NT - 1) * 128 // 512 + 1):
                        ensure_qk(2 + h, b)
                    for t in range(mt, min(mt + VLOOK, NT)):
                        ensure_v(t)

                idx = 0
                for h in range(2):
                    for nw in range(NBLK):
                        for mt in range(NT):
                            one_tile(idx, h, nw, mt)
                            idx += 1

            def body(_i=None):
                qk_done.clear()
                v_done.clear()
                fill_state["i"] = 0
                fill_state["tick"] = 0
                pending.clear()
                attn_nw_stream()
                for p in pending:
                    emit_pv(*p)
                    pop_filler()
                pending.clear()
                # backstop: anything the filler didn't reach
                for j, b in filler:
                    ensure_qk(j, b)

            if loop_iters == 1:
                body()
            else:
                with tc.For_i(0, loop_iters, 1) as _i:
                    body(_i)

    nc.compile()
    return nc


def get_program(loop_iters=1, variant="full"):
    key = ("nc", loop_iters, variant)
    if key not in _CACHE:
        _CACHE[key] = build_program(loop_iters, variant)
    return _CACHE[key]


def make_in_maps(x, W_qkv, b_qkv):
    x = np.asarray(x, np.float32)
    W = np.asarray(W_qkv, np.float32)
    b = np.asarray(b_qkv, np.float32)
    Wq, Wk, Wv = W[:, :DIM], W[:, DIM:2 * DIM], W[:, 2 * DIM:]
    bq, bk, bv = b[:DIM], b[DIM:2 * DIM], b[2 * DIM:]

    in_maps = []
    for c in range(NCORES):
        bb, hp = divmod(c, 4)
        h0 = 2 * hp
        s = slice(h0 * DH, (h0 + 1) * DH)
        s1 = slice((h0 + 1) * DH, (h0 + 2) * DH)
        xT = np.ascontiguousarray(x[bb].T).astype(np.float16)
        wqk = np.concatenate(
            [Wq[:, s], Wq[:, s1], Wk[:, s], Wk[:, s1]], axis=1
        ).astype(np.float16)
        bqk96 = np.stack([bq[s], bq[s1], bk[s], bk[s1]], axis=1).astype(
            np.float32
        )
        wv = np.zeros((DIM, VW), np.float16)
        wv[:, 0:DH] = Wv[:, s].astype(np.float16)
        wv[:, DH + 1:2 * DH + 1] = Wv[:, s1].astype(np.float16)
        in_maps.append(
            {"xT": xT, "wqk": wqk, "bqk96": bqk96, "wv": wv}
        )
    return in_maps


def gather_out(results, b_qkv):
    bv = np.asarray(b_qkv, np.float32)[2 * DIM:]
    out = np.empty((B, N, DIM), np.float32)
    for c in range(NCORES):
        bb, hp = divmod(c, 4)
        o = np.asarray(results[c]["out"], np.float32)  # [2, NBLK, 97, 512]
        # out[h, nw, d, c] = numerator dim d (d<96) / denominator (d=96)
        # for token nw*512 + c
        num = o[:, :, :DH, :]                 # [2, NBLK, 96, 512]
        den = o[:, :, DH:DH + 1, :]           # [2, NBLK, 1, 512]
        r = (num / den).transpose(0, 1, 3, 2).reshape(2, N, DH)
        h0 = 2 * hp
        out[bb, :, h0 * DH:(h0 + 1) * DH] = r[0] + bv[h0 * DH:(h0 + 1) * DH]
        out[bb, :, (h0 + 1) * DH:(h0 + 2) * DH] = (
            r[1] + bv[(h0 + 1) * DH:(h0 + 2) * DH])
    return out


def run(x, W_qkv, b_qkv, trace=False, **kw):
    from concourse.bass_utils import run_bass_kernel_spmd

    nc = get_program()
    in_maps = make_in_maps(x, W_qkv, b_qkv)
    res = run_bass_kernel_spmd(nc, in_maps, list(range(NCORES)), trace=trace, **kw)
    return gather_out(res.results, b_qkv), res


def kernel(x, W_qkv, b_qkv):
    out, _ = run(x, W_qkv, b_qkv)
    return out
